# revision 8
# baseline (speedup 1.0000x reference)
"""Trainium2 Bass kernel for CohereAttention (QK-LayerNorm + interleaved RoPE +
GQA sliding-window attention), sharded over 8 NeuronCores.

Sharding: tensor-parallel over Q heads (4 per core); with H//KVH == 4 each core
owns exactly one KV head. o_proj is ROW-parallel: each core contracts its own
512 attention-output features (read straight from SBUF) against its 512 rows of
wo, producing a full [4096, S] partial that a chunked bf16 ReduceScatter sums;
core c ends with output-feature rows [2048k+256c, +256) for chunk k.

Device-side layouts are transposed ([feature, token]) so every matmul contracts
over the partition axis at full PE rate:
  - QK-LayerNorm mean subtraction is folded into the projection weights on the
    host (subtract per-head column mean), leaving an RMS-style normalization.
  - RoPE rotate-half is a partition pair-swap (DVE stream_shuffle) with the sign
    folded into the sin table on the host.
  - Scores are computed transposed (S^T[j, q]) with QT=512 query tiles; the
    sliding-window/causal mask is a DVE multiply against slices of one
    precomputed [128, 1408] band mask; softmax denominator comes from a
    ones-vector matmul and is applied at the attention-output drain.
  - Score matmuls are emitted one chunk ahead of the PV matmuls so the tensor
    queue never stalls on the exp/mask chain.
"""

import sys

sys.path.insert(0, "/opt/trn_rl_repo")

import numpy as np
import ml_dtypes

import concourse.bass as bass
import concourse.mybir as mybir
import concourse.tile as tile
from concourse import bacc
from concourse.bass import ts, ds
from concourse.bass_utils import run_bass_kernel_spmd

B, S, H, KVH, D, HID = 2, 2048, 32, 8, 128, 4096
WINDOW = 512
EPS = 1e-5
SCALE = float(D) ** -0.5
NC = 8
HPC = H // NC              # q heads per core (4)
QW = HPC * D               # q width per core (512)
FCH = HID // 128           # contraction chunks (32)
TT = 512                   # projection token tile
QT = 512                   # attention query tile
NOC = HID // 128           # o_proj output-feature chunks (32)
RSCH = 2                   # ReduceScatter chunks per batch
RSROWS = HID // RSCH       # rows per RS chunk (2048)
OROWS = RSROWS // NC       # rows per core per RS chunk (256)

BF16 = mybir.dt.bfloat16
F32 = mybir.dt.float32
npbf16 = ml_dtypes.bfloat16

SWAP32 = [i ^ 1 for i in range(32)]  # adjacent-pair partition swap

_CACHE = {}


def _band_mask():
    # M[jj, u] = 1 iff 0 <= (u - 384) - jj < 512; chunk kk uses cols
    # [896-128kk, 896-128kk+512) so the slice keeps 0 <= i-j < WINDOW.
    jj = np.arange(128)[:, None]
    u = np.arange(1408)[None, :]
    d = u - 384 - jj
    return ((d >= 0) & (d < WINDOW)).astype(npbf16)


def _build_module():
    nc = bacc.Bacc(
        "TRN2",
        target_bir_lowering=False,
        debug=False,
        enable_asserts=False,
        num_devices=NC,
    )

    hT = nc.dram_tensor("hT", [B, HID, S], BF16, kind="ExternalInput").ap()
    cosT = nc.dram_tensor("cosT", [B, D, S], BF16, kind="ExternalInput").ap()
    sinT = nc.dram_tensor("sinT", [B, D, S], BF16, kind="ExternalInput").ap()
    wq = nc.dram_tensor("wq", [HID, QW], BF16, kind="ExternalInput").ap()
    wk = nc.dram_tensor("wk", [HID, D], BF16, kind="ExternalInput").ap()
    wv = nc.dram_tensor("wv", [HID, D], BF16, kind="ExternalInput").ap()
    # host-tiled: wo[p, idx//4, (idx%4)*128+n] = wo_rows[h*128+p, o*128+n]
    # with idx = h*32+o; shaped [128, FCH, QW] so it can reuse wq's SBUF slot
    wo = nc.dram_tensor("wo", [128, FCH, QW], BF16, kind="ExternalInput").ap()
    winvq = nc.dram_tensor("winvq", [D, 1], F32, kind="ExternalInput").ap()
    winvk = nc.dram_tensor("winvk", [D, 1], F32, kind="ExternalInput").ap()
    out = nc.dram_tensor("out", [B, RSCH, OROWS, S], BF16, kind="ExternalOutput").ap()

    rsin = [
        nc.dram_tensor(f"rsin{b}", [HID, S], BF16, kind="Internal").ap()
        for b in range(B)
    ]
    rsout = [
        [
            nc.dram_tensor(f"rsout{b}_{k}", [OROWS, S], BF16, kind="Internal").ap()
            for k in range(RSCH)
        ]
        for b in range(B)
    ]

    ident_d = nc.inline_tensor(np.eye(128, dtype=npbf16), name="ident").ap()
    ones_d = nc.inline_tensor(np.ones((128, 1), dtype=npbf16), name="onesv").ap()
    mask_d = nc.inline_tensor(_band_mask(), name="bandmask").ap()

    rg = [list(range(NC))]

    with tile.TileContext(nc) as tc, \
            tc.tile_pool(name="sb", bufs=1) as sb, \
            tc.tile_pool(name="ps", bufs=1, space="PSUM") as ps:

        # --- resident weights / constants (wq chunked so proj starts early) ---
        wq_sb = sb.tile([128, FCH, QW], BF16, tag="wq", bufs=1, name="wq_sb")
        wqr = wq.rearrange("(c p) n -> p c n", p=128)
        for j in range(4):
            nc.sync.dma_start(wq_sb[:, ds(8 * j, 8), :], wqr[:, ds(8 * j, 8), :])
        wk_sb = sb.tile([128, FCH, D], BF16, tag="wk", bufs=1, name="wk_sb")
        nc.sync.dma_start(wk_sb[:], wk.rearrange("(c p) n -> p c n", p=128))
        wv_sb = sb.tile([128, FCH, D], BF16, tag="wv", bufs=1, name="wv_sb")
        nc.sync.dma_start(wv_sb[:], wv.rearrange("(c p) n -> p c n", p=128))
        ident_sb = sb.tile([128, 128], BF16, tag="ident", bufs=1, name="ident_sb")
        nc.sync.dma_start(ident_sb[:], ident_d)
        ones_sb = sb.tile([128, 1], BF16, tag="ones", bufs=1, name="ones_sb")
        nc.sync.dma_start(ones_sb[:], ones_d)
        mask_sb = sb.tile([128, 1408], BF16, tag="mask", bufs=1, name="mask_sb")
        nc.sync.dma_start(mask_sb[:], mask_d)
        winvq_sb = sb.tile([D, 1], F32, tag="winvq", bufs=1, name="winvq_sb")
        nc.sync.dma_start(winvq_sb[:], winvq)
        winvk_sb = sb.tile([D, 1], F32, tag="winvk", bufs=1, name="winvk_sb")
        nc.sync.dma_start(winvk_sb[:], winvk)
        eps_sb = sb.tile([1, 1], F32, tag="eps", bufs=1, name="eps_sb")
        nc.vector.memset(eps_sb[:], EPS)

        def ln_rope(qps, winv_sb, cos_sb, sin_sb, tt, dst):
            """LayerNorm (mean pre-folded) + interleaved RoPE on a transposed
            [d, TT] psum tile; writes bf16 into dst[:, tt*TT:...]."""
            sq = sb.tile([128, TT], F32, tag="sq", bufs=2, name="sq")
            nc.scalar.square(sq[:], qps[:])
            qsb = sb.tile([128, TT], F32, tag="qsb", bufs=2, name="qsb")
            nc.scalar.copy(qsb[:], qps[:])  # frees the psum bank early
            ssq = ps.tile([1, TT], F32, tag="misc", bufs=2, name="ssq")
            nc.tensor.matmul(ssq[:], winv_sb[:], sq[:], start=True, stop=True)
            std = sb.tile([1, TT], F32, tag="std", bufs=2, name="std")
            nc.scalar.activation(
                std[:], ssq[:], mybir.ActivationFunctionType.Sqrt,
                bias=eps_sb[:], scale=1.0 / D,
            )
            rstd = sb.tile([1, TT], F32, tag="rstd", bufs=2, name="rstd")
            nc.vector.reciprocal(rstd[:], std[:])
            rbc = sb.tile([128, TT], F32, tag="rbc", bufs=2, name="rbc")
            nc.gpsimd.partition_broadcast(rbc[:], rstd[:])
            qn = sb.tile([128, TT], BF16, tag="qn", bufs=2, name="qn")
            nc.vector.tensor_mul(qn[:], qsb[:], rbc[:])
            qs = sb.tile([128, TT], BF16, tag="qs", bufs=2, name="qs")
            nc.vector.stream_shuffle(qs[:], qn[:], SWAP32)
            t1 = sb.tile([128, TT], BF16, tag="t1", bufs=2, name="t1")
            nc.vector.tensor_mul(t1[:], qn[:], cos_sb[:, ts(tt, TT)])
            t2 = sb.tile([128, TT], BF16, tag="t2", bufs=2, name="t2")
            nc.vector.tensor_mul(t2[:], qs[:], sin_sb[:, ts(tt, TT)])
            nc.vector.tensor_add(dst[:, ts(tt, TT)], t1[:], t2[:])

        qT = {}   # (b, h) -> [128, S] bf16 rope'd normalized q, transposed
        kT = {}   # b -> [128, S]
        Vn = {}   # b -> [128, S] (natural [j, d] in 128-col chunks)
        asb = {}  # (b, h) -> [128, S] bf16 attention output (d, token)

        def proj(b):
            with nc.named_scope(f"proj_b{b}"):
                cos_sb = sb.tile([128, S], BF16, tag="cos", bufs=1, name="cos_sb")
                nc.sync.dma_start(cos_sb[:], cosT[b])
                sin_sb = sb.tile([128, S], BF16, tag="sin", bufs=1, name="sin_sb")
                nc.sync.dma_start(sin_sb[:], sinT[b])
                for h in range(HPC):
                    qT[(b, h)] = sb.tile([128, S], BF16, tag="qT", bufs=8,
                                         name=f"qT{b}{h}")
                kT[b] = sb.tile([128, S], BF16, tag="kT", bufs=2, name=f"kT{b}")
                vT = sb.tile([128, S], BF16, tag="vT", bufs=1, name=f"vT{b}")
                Vn[b] = sb.tile([128, S], BF16, tag="Vn", bufs=2, name=f"Vn{b}")
                hTr = hT[b].rearrange("(c p) s -> p c s", p=128)
                for tt in range(S // TT):
                    qps = [
                        ps.tile([128, TT], F32, tag=f"pb{i}", bufs=1,
                                name=f"qps{i}")
                        for i in range(HPC)
                    ]
                    kps = ps.tile([128, TT], F32, tag="pb4", bufs=1, name="kps")
                    vps = ps.tile([128, TT], F32, tag="pb5", bufs=1, name="vps")
                    for f2 in range(FCH // 2):
                        ht_t = sb.tile([128, 2, TT], BF16, tag="ht", bufs=6,
                                       name="ht_t")
                        nc.sync.dma_start(
                            ht_t[:], hTr[:, ds(2 * f2, 2), ts(tt, TT)]
                        )
                        for s2 in range(2):
                            f = 2 * f2 + s2
                            st = f == 0
                            sp = f == FCH - 1
                            for h in range(HPC):
                                nc.tensor.matmul(
                                    qps[h][:], wq_sb[:, f, ts(h, D)],
                                    ht_t[:, s2], start=st, stop=sp,
                                )
                            nc.tensor.matmul(kps[:], wk_sb[:, f, :],
                                             ht_t[:, s2], start=st, stop=sp)
                            nc.tensor.matmul(vps[:], wv_sb[:, f, :],
                                             ht_t[:, s2], start=st, stop=sp)
                    for h in range(HPC):
                        ln_rope(qps[h], winvq_sb, cos_sb, sin_sb, tt, qT[(b, h)])
                    ln_rope(kps, winvk_sb, cos_sb, sin_sb, tt, kT[b])
                    nc.scalar.copy(vT[:, ts(tt, TT)], vps[:])
                # transpose v to natural [j, d] layout for the PV matmul
                for j in range(S // 128):
                    tp = ps.tile([128, 128], BF16, tag="misc", bufs=2, name="tp")
                    nc.tensor.transpose(tp[:], vT[:, ts(j, 128)], ident_sb[:])
                    nc.scalar.copy(Vn[b][:, ts(j, 128)], tp[:])

        def attn(b):
            with nc.named_scope(f"attn_b{b}"):
                for h in range(HPC):
                    a = sb.tile([128, S], BF16, tag="attn", bufs=4,
                                name=f"attn{b}{h}")
                    asb[(b, h)] = a
                    for qt in range(S // QT):
                        i0 = qt * QT
                        if qt == 0:
                            chunks = [(128 * m, 4 + m) for m in range(4)]
                        else:
                            chunks = [(i0 - WINDOW + 128 * k, k)
                                      for k in range(8)]
                        n = len(chunks)
                        ops = ps.tile([128, QT], F32, tag=f"pb{2 + qt % 2}",
                                      bufs=1, name="ops")
                        lps = ps.tile([1, QT], F32, tag="misc", bufs=2,
                                      name="lps")
                        pts = []

                        def score_chain(idx):
                            j0, kke = chunks[idx]
                            sps = ps.tile([128, QT], F32, tag=f"pb{idx % 2}",
                                          bufs=1, name="sps")
                            nc.tensor.matmul(
                                sps[:], kT[b][:, ds(j0, 128)],
                                qT[(b, h)][:, ds(i0, QT)],
                                start=True, stop=True,
                            )
                            praw = sb.tile([128, QT], BF16, tag="praw", bufs=4,
                                           name="praw")
                            nc.scalar.activation(
                                praw[:], sps[:],
                                mybir.ActivationFunctionType.Exp, scale=SCALE,
                            )
                            pt = sb.tile([128, QT], BF16, tag="pt", bufs=4,
                                         name="pt")
                            nc.vector.tensor_mul(
                                pt[:], praw[:],
                                mask_sb[:, ds(896 - 128 * kke, QT)],
                            )
                            pts.append(pt)

                        def pv(idx):
                            j0, _ = chunks[idx]
                            st = idx == 0
                            sp = idx == n - 1
                            nc.tensor.matmul(ops[:], Vn[b][:, ds(j0, 128)],
                                             pts[idx][:], start=st, stop=sp)
                            nc.tensor.matmul(lps[:], ones_sb[:], pts[idx][:],
                                             start=st, stop=sp)

                        score_chain(0)
                        for idx in range(1, n):
                            score_chain(idx)
                            pv(idx - 1)
                        pv(n - 1)
                        linv = sb.tile([1, QT], F32, tag="linv", bufs=2,
                                       name="linv")
                        nc.vector.reciprocal(linv[:], lps[:])
                        lbc = sb.tile([128, QT], F32, tag="lbc", bufs=2,
                                      name="lbc")
                        nc.gpsimd.partition_broadcast(lbc[:], linv[:])
                        nc.vector.tensor_mul(a[:, ds(i0, QT)], ops[:], lbc[:])

        def oproj(b, wo_sb):
            with nc.named_scope(f"oproj_b{b}"):
                for half in range(RSCH):
                    for oc in range(NOC // RSCH):
                        o = half * (NOC // RSCH) + oc
                        for tk in range(S // 512):
                            i = oc * 4 + tk
                            po = ps.tile([128, 512], F32, tag=f"pb{4 + i % 2}",
                                         bufs=1, name="po")
                            for h in range(HPC):
                                idx = h * NOC + o
                                nc.tensor.matmul(
                                    po[:],
                                    wo_sb[:, idx // 4, ds((idx % 4) * 128, 128)],
                                    asb[(b, h)][:, ts(tk, 512)],
                                    start=(h == 0), stop=(h == HPC - 1),
                                )
                            ot = sb.tile([128, 512], BF16, tag="ot", bufs=6,
                                         name="ot")
                            if i % 2 == 0:
                                nc.scalar.copy(ot[:], po[:])
                            else:
                                nc.vector.tensor_copy(ot[:], po[:])
                            nc.sync.dma_start(
                                rsin[b][ds(o * 128, 128), ts(tk, 512)], ot[:]
                            )
                    nc.gpsimd.collective_compute(
                        "ReduceScatter",
                        mybir.AluOpType.add,
                        replica_groups=rg,
                        ins=[rsin[b][ds(half * RSROWS, RSROWS), :]],
                        outs=[rsout[b][half][:]],
                    )

        def outcopy(b):
            for half in range(RSCH):
                for r in range(OROWS // 128):
                    t = sb.tile([128, S], BF16, tag="oc", bufs=2, name="oc")
                    nc.sync.dma_start(t[:], rsout[b][half][ds(r * 128, 128), :])
                    nc.sync.dma_start(out[b, half, ds(r * 128, 128), :], t[:])

        proj(0)
        proj(1)
        # wo load rides the DMA-free attention phase; reuses wq's SBUF slot
        # (wq is dead once proj(1) has drained)
        wo_sb = sb.tile([128, FCH, QW], BF16, tag="wq", bufs=1, name="wo_sb")
        nc.sync.dma_start(wo_sb[:], wo)
        attn(0)
        oproj(0, wo_sb)
        attn(1)
        outcopy(0)
        oproj(1, wo_sb)
        outcopy(1)

    nc.compile()
    return nc


def _prep_inputs(inputs):
    hidden = np.asarray(inputs["hidden_states"], np.float32)
    pos = np.asarray(inputs["position_ids"])
    cos = np.asarray(inputs["cos"], np.float32)
    sin = np.asarray(inputs["sin"], np.float32)
    wq = np.asarray(inputs["wq"], np.float32)
    wk = np.asarray(inputs["wk"], np.float32)
    wv = np.asarray(inputs["wv"], np.float32)
    wo = np.asarray(inputs["wo"], np.float32)
    qw = np.asarray(inputs["q_norm_w"], np.float32)
    kw = np.asarray(inputs["k_norm_w"], np.float32)

    hT = np.ascontiguousarray(hidden.transpose(0, 2, 1)).astype(npbf16)
    cosT = np.ascontiguousarray(cos[pos].transpose(0, 2, 1)).astype(npbf16)
    sinT_f = sin[pos].transpose(0, 2, 1).copy()
    sinT_f[:, 0::2, :] *= -1.0
    sinT = np.ascontiguousarray(sinT_f).astype(npbf16)

    winvq = (1.0 / np.where(qw == 0, 1, qw) ** 2).astype(np.float32).reshape(D, 1)
    winvk = (1.0 / np.where(kw == 0, 1, kw) ** 2).astype(np.float32).reshape(D, 1)

    in_maps = []
    for c in range(NC):
        wq_c = wq[:, c * QW:(c + 1) * QW].copy()
        for j in range(HPC):
            blk = wq_c[:, j * D:(j + 1) * D]
            blk -= blk.mean(axis=1, keepdims=True)
            blk *= qw[None, :]
        wk_c = wk[:, c * D:(c + 1) * D].copy()
        wk_c -= wk_c.mean(axis=1, keepdims=True)
        wk_c *= kw[None, :]
        wo_c = wo[c * QW:(c + 1) * QW, :]
        wo_t = np.ascontiguousarray(
            wo_c.reshape(HPC, 128, NOC, 128).transpose(1, 0, 2, 3)
            .reshape(128, FCH, QW)
        ).astype(npbf16)
        in_maps.append({
            "hT": hT,
            "cosT": cosT,
            "sinT": sinT,
            "wq": np.ascontiguousarray(wq_c).astype(npbf16),
            "wk": np.ascontiguousarray(wk_c).astype(npbf16),
            "wv": np.ascontiguousarray(wv[:, c * D:(c + 1) * D]).astype(npbf16),
            "wo": wo_t,
            "winvq": winvq,
            "winvk": winvk,
        })
    return in_maps


def _run(inputs, **kwargs):
    if "nc" not in _CACHE:
        _CACHE["nc"] = _build_module()
    nc = _CACHE["nc"]
    in_maps = _prep_inputs(inputs)
    res = run_bass_kernel_spmd(nc, in_maps, core_ids=list(range(NC)), **kwargs)
    # core c returns out[b, k, :, :] = rows [2048k + 256c, +256) of outT[b]
    outT = np.empty((B, HID, S), np.float32)
    for c in range(NC):
        shard = np.asarray(res.results[c]["out"], dtype=np.float32)
        for k in range(RSCH):
            outT[:, k * RSROWS + c * OROWS: k * RSROWS + (c + 1) * OROWS, :] = \
                shard[:, k]
    full = np.ascontiguousarray(outT.transpose(0, 2, 1))
    return full, res


def kernel(**inputs) -> np.ndarray:
    out, _ = _run(inputs)
    return out


if __name__ == "__main__":
    import reference
    ins = {k: np.asarray(v) for k, v in reference.setup_inputs().items()}
    expected = np.asarray(reference.reference(**reference.setup_inputs()))
    actual = kernel(**ins)
    err = np.linalg.norm(actual - expected) / np.linalg.norm(expected)
    print("Relative error:", err)


# revision 16
# speedup vs baseline: 1.1453x; 1.1453x over previous
"""Trainium2 Bass kernel for CohereAttention (QK-LayerNorm + interleaved RoPE +
GQA sliding-window attention), sharded over 8 NeuronCores.

Sharding: tensor-parallel over Q heads (4 per core); with H//KVH == 4 each core
owns exactly one KV head. o_proj is ROW-parallel: each core contracts its own
512 attention-output features (read straight from SBUF) against its 512 rows of
wo, producing a full [4096, S] partial that a chunked bf16 ReduceScatter sums;
core c ends with output-feature rows [2048k+256c, +256) for chunk k.

Device-side layouts are transposed ([feature, token]) so every matmul contracts
over the partition axis at full PE rate:
  - QK-LayerNorm mean subtraction is folded into the projection weights on the
    host (subtract per-head column mean), leaving an RMS-style normalization.
  - RoPE rotate-half is a partition pair-swap (DVE stream_shuffle) with the sign
    folded into the sin table on the host.
  - Scores are computed transposed (S^T[j, q]) with QT=512 query tiles; the
    sliding-window/causal mask is a DVE multiply against slices of one
    precomputed [128, 1408] band mask; softmax denominator comes from a
    ones-vector matmul and is applied at the attention-output drain.
  - Score matmuls are emitted one chunk ahead of the PV matmuls so the tensor
    queue never stalls on the exp/mask chain.
"""

import sys

sys.path.insert(0, "/opt/trn_rl_repo")

import numpy as np
import ml_dtypes

import concourse.bass as bass
import concourse.mybir as mybir
import concourse.tile as tile
from concourse import bacc
from concourse.bass import ts, ds
from concourse.bass_utils import run_bass_kernel_spmd

B, S, H, KVH, D, HID = 2, 2048, 32, 8, 128, 4096
WINDOW = 512
EPS = 1e-5
SCALE = float(D) ** -0.5
NC = 8
HPC = H // NC              # q heads per core (4)
QW = HPC * D               # q width per core (512)
FCH = HID // 128           # contraction chunks (32)
TT = 512                   # projection token tile
QT = 512                   # attention query tile
NOC = HID // 128           # o_proj output-feature chunks (32)
# ReduceScatter chunks per batch: (row0, nrows), finer at the end of b1 so the
# exposed tail collective is small
RS_CHUNKS = [
    [(0, 2048), (2048, 2048)],
    [(0, 2048), (2048, 1024), (3072, 1024)],
]

BF16 = mybir.dt.bfloat16
F32 = mybir.dt.float32
npbf16 = ml_dtypes.bfloat16

SWAP32 = [i ^ 1 for i in range(32)]  # adjacent-pair partition swap

_CACHE = {}


def _band_mask():
    # M[jj, u] = 1 iff 0 <= (u - 384) - jj < 512; chunk kk uses cols
    # [896-128kk, 896-128kk+512) so the slice keeps 0 <= i-j < WINDOW.
    jj = np.arange(128)[:, None]
    u = np.arange(1408)[None, :]
    d = u - 384 - jj
    return ((d >= 0) & (d < WINDOW)).astype(npbf16)


def _build_module():
    nc = bacc.Bacc(
        "TRN2",
        target_bir_lowering=False,
        debug=False,
        enable_asserts=False,
        num_devices=NC,
    )

    hT = nc.dram_tensor("hT", [B, HID, S], BF16, kind="ExternalInput").ap()
    cosT = nc.dram_tensor("cosT", [B, D, S], BF16, kind="ExternalInput").ap()
    sinT = nc.dram_tensor("sinT", [B, D, S], BF16, kind="ExternalInput").ap()
    wq = nc.dram_tensor("wq", [HID, QW], BF16, kind="ExternalInput").ap()
    wk = nc.dram_tensor("wk", [HID, D], BF16, kind="ExternalInput").ap()
    wv = nc.dram_tensor("wv", [HID, D], BF16, kind="ExternalInput").ap()
    # host-tiled: wo[p, idx//4, (idx%4)*128+n] = wo_rows[h*128+p, o*128+n]
    # with idx = h*32+o; shaped [128, FCH, QW] so it can reuse wq's SBUF slot
    wo = nc.dram_tensor("wo", [128, FCH, QW], BF16, kind="ExternalInput").ap()
    winvq = nc.dram_tensor("winvq", [D, 1], F32, kind="ExternalInput").ap()
    winvk = nc.dram_tensor("winvk", [D, 1], F32, kind="ExternalInput").ap()
    # per-core output rows: 512 of-rows per batch, concatenated over RS chunks
    out = nc.dram_tensor("out", [B, QW, S], BF16, kind="ExternalOutput").ap()

    rsin = [
        nc.dram_tensor(f"rsin{b}", [HID, S], BF16, kind="Internal").ap()
        for b in range(B)
    ]
    rsout = [
        [
            nc.dram_tensor(f"rsout{b}_{k}", [nr // NC, S], BF16,
                           kind="Internal").ap()
            for k, (r0, nr) in enumerate(RS_CHUNKS[b])
        ]
        for b in range(B)
    ]

    ident_d = nc.inline_tensor(np.eye(128, dtype=npbf16), name="ident").ap()
    ones_d = nc.inline_tensor(np.ones((128, 1), dtype=npbf16), name="onesv").ap()
    mask_d = nc.inline_tensor(_band_mask(), name="bandmask").ap()

    rg = [list(range(NC))]

    with tile.TileContext(nc) as tc, \
            tc.tile_pool(name="sb", bufs=1) as sb, \
            tc.tile_pool(name="ps", bufs=1, space="PSUM") as ps:

        # --- resident weights / constants (wq chunked so proj starts early) ---
        wq_sb = sb.tile([128, FCH, QW], BF16, tag="wq", bufs=1, name="wq_sb")
        wqr = wq.rearrange("(c p) n -> p c n", p=128)
        for j in range(4):
            nc.sync.dma_start(wq_sb[:, ds(8 * j, 8), :], wqr[:, ds(8 * j, 8), :])
        wk_sb = sb.tile([128, FCH, D], BF16, tag="wk", bufs=1, name="wk_sb")
        nc.sync.dma_start(wk_sb[:], wk.rearrange("(c p) n -> p c n", p=128))
        wv_sb = sb.tile([128, FCH, D], BF16, tag="wv", bufs=1, name="wv_sb")
        nc.sync.dma_start(wv_sb[:], wv.rearrange("(c p) n -> p c n", p=128))
        ident_sb = sb.tile([128, 128], BF16, tag="ident", bufs=1, name="ident_sb")
        nc.sync.dma_start(ident_sb[:], ident_d)
        ones_sb = sb.tile([128, 1], BF16, tag="ones", bufs=1, name="ones_sb")
        nc.sync.dma_start(ones_sb[:], ones_d)
        mask_sb = sb.tile([128, 1408], BF16, tag="mask", bufs=1, name="mask_sb")
        nc.sync.dma_start(mask_sb[:], mask_d)
        winvq_sb = sb.tile([D, 1], F32, tag="winvq", bufs=1, name="winvq_sb")
        nc.sync.dma_start(winvq_sb[:], winvq)
        winvk_sb = sb.tile([D, 1], F32, tag="winvk", bufs=1, name="winvk_sb")
        nc.sync.dma_start(winvk_sb[:], winvk)
        eps_sb = sb.tile([1, 1], F32, tag="eps", bufs=1, name="eps_sb")
        nc.vector.memset(eps_sb[:], EPS)

        def ln_rope(qps, winv_sb, cos_sb, sin_sb, tt, dst):
            """LayerNorm (mean pre-folded) + interleaved RoPE on a transposed
            [d, TT] psum tile; writes bf16 into dst[:, tt*TT:...]."""
            sq = sb.tile([128, TT], F32, tag="sq", bufs=2, name="sq")
            nc.scalar.square(sq[:], qps[:])
            qsb = sb.tile([128, TT], F32, tag="qsb", bufs=2, name="qsb")
            nc.vector.tensor_copy(qsb[:], qps[:])  # frees the psum bank early
            ssq = ps.tile([1, TT], F32, tag="misc", bufs=2, name="ssq")
            nc.tensor.matmul(ssq[:], winv_sb[:], sq[:], start=True, stop=True)
            std = sb.tile([1, TT], F32, tag="std", bufs=2, name="std")
            nc.scalar.activation(
                std[:], ssq[:], mybir.ActivationFunctionType.Sqrt,
                bias=eps_sb[:], scale=1.0 / D,
            )
            rstd = sb.tile([1, TT], F32, tag="rstd", bufs=2, name="rstd")
            nc.vector.reciprocal_approx_fast(rstd[:], std[:])
            rbc = sb.tile([128, TT], F32, tag="rbc", bufs=2, name="rbc")
            nc.gpsimd.partition_broadcast(rbc[:], rstd[:])
            qn = sb.tile([128, TT], BF16, tag="qn", bufs=2, name="qn")
            nc.vector.tensor_mul(qn[:], qsb[:], rbc[:])
            qs = sb.tile([128, TT], BF16, tag="qs", bufs=2, name="qs")
            nc.vector.stream_shuffle(qs[:], qn[:], SWAP32)
            t1 = sb.tile([128, TT], BF16, tag="t1", bufs=2, name="t1")
            nc.vector.tensor_mul(t1[:], qn[:], cos_sb[:, ts(tt, TT)])
            t2 = sb.tile([128, TT], BF16, tag="t2", bufs=2, name="t2")
            nc.vector.tensor_mul(t2[:], qs[:], sin_sb[:, ts(tt, TT)])
            nc.vector.tensor_add(dst[:, ts(tt, TT)], t1[:], t2[:])

        qT = {}   # (b, h) -> [128, S] bf16 rope'd normalized q, transposed
        kT = {}   # b -> [128, S]
        Vn = {}   # b -> [128, S] (natural [j, d] in 128-col chunks)
        asb = {}  # (b, h) -> [128, S] bf16 attention output (d, token)

        def proj(b):
            with nc.named_scope(f"proj_b{b}"):
                cos_sb = sb.tile([128, S], BF16, tag="cos", bufs=1, name="cos_sb")
                nc.sync.dma_start(cos_sb[:], cosT[b])
                sin_sb = sb.tile([128, S], BF16, tag="sin", bufs=1, name="sin_sb")
                nc.sync.dma_start(sin_sb[:], sinT[b])
                for h in range(HPC):
                    qT[(b, h)] = sb.tile([128, S], BF16, tag="qT", bufs=8,
                                         name=f"qT{b}{h}")
                kT[b] = sb.tile([128, S], BF16, tag="kT", bufs=2, name=f"kT{b}")
                vT = sb.tile([128, S], BF16, tag="vT", bufs=1, name=f"vT{b}")
                Vn[b] = sb.tile([128, S], BF16, tag="Vn", bufs=2, name=f"Vn{b}")
                hTr = hT[b].rearrange("(c p) s -> p c s", p=128)
                for tt in range(S // TT):
                    qps = [
                        ps.tile([128, TT], F32, tag=f"pb{i}", bufs=1,
                                name=f"qps{i}")
                        for i in range(HPC)
                    ]
                    kps = ps.tile([128, TT], F32, tag="pb4", bufs=1, name="kps")
                    vps = ps.tile([128, TT], F32, tag="pb5", bufs=1, name="vps")
                    for f2 in range(FCH // 2):
                        ht_t = sb.tile([128, 2, TT], BF16, tag="ht", bufs=4,
                                       name="ht_t")
                        nc.sync.dma_start(
                            ht_t[:], hTr[:, ds(2 * f2, 2), ts(tt, TT)]
                        )
                        for s2 in range(2):
                            f = 2 * f2 + s2
                            st = f == 0
                            sp = f == FCH - 1
                            for h in range(HPC):
                                nc.tensor.matmul(
                                    qps[h][:], wq_sb[:, f, ts(h, D)],
                                    ht_t[:, s2], start=st, stop=sp,
                                )
                            nc.tensor.matmul(kps[:], wk_sb[:, f, :],
                                             ht_t[:, s2], start=st, stop=sp)
                            nc.tensor.matmul(vps[:], wv_sb[:, f, :],
                                             ht_t[:, s2], start=st, stop=sp)
                    for h in range(HPC):
                        ln_rope(qps[h], winvq_sb, cos_sb, sin_sb, tt, qT[(b, h)])
                    ln_rope(kps, winvk_sb, cos_sb, sin_sb, tt, kT[b])
                    nc.scalar.copy(vT[:, ts(tt, TT)], vps[:])
                # transpose v to natural [j, d] layout for the PV matmul
                for j in range(S // 128):
                    tp = ps.tile([128, 128], BF16, tag="misc", bufs=2, name="tp")
                    nc.tensor.transpose(tp[:], vT[:, ts(j, 128)], ident_sb[:])
                    nc.scalar.copy(Vn[b][:, ts(j, 128)], tp[:])

        def attn(b):
            with nc.named_scope(f"attn_b{b}"):
                for h in range(HPC):
                    a = sb.tile([128, S], BF16, tag="attn", bufs=4,
                                name=f"attn{b}{h}")
                    asb[(b, h)] = a
                    for qt in range(S // QT):
                        i0 = qt * QT
                        if qt == 0:
                            chunks = [(128 * m, 4 + m) for m in range(4)]
                        else:
                            chunks = [(i0 - WINDOW + 128 * k, k)
                                      for k in range(8)]
                        n = len(chunks)
                        ops = ps.tile([128, QT], F32, tag=f"pb{2 + qt % 2}",
                                      bufs=1, name="ops")
                        lps = ps.tile([1, QT], F32, tag="misc", bufs=2,
                                      name="lps")
                        pts = []

                        def score_chain(idx):
                            j0, kke = chunks[idx]
                            sps = ps.tile([128, QT], F32, tag=f"pb{idx % 2}",
                                          bufs=1, name="sps")
                            nc.tensor.matmul(
                                sps[:], kT[b][:, ds(j0, 128)],
                                qT[(b, h)][:, ds(i0, QT)],
                                start=True, stop=True,
                            )
                            praw = sb.tile([128, QT], BF16, tag="praw", bufs=4,
                                           name="praw")
                            nc.scalar.activation(
                                praw[:], sps[:],
                                mybir.ActivationFunctionType.Exp, scale=SCALE,
                            )
                            pt = sb.tile([128, QT], BF16, tag="pt", bufs=4,
                                         name="pt")
                            # mask alternates gpsimd/DVE to balance engine load
                            if idx % 2 == 0:
                                if kke < 4:  # window edge: qi-jj <= 128kke-1
                                    nc.gpsimd.affine_select(
                                        out=pt[:], in_=praw[:],
                                        compare_op=mybir.AluOpType.is_ge,
                                        fill=0.0, base=128 * kke - 1,
                                        channel_multiplier=1,
                                        pattern=[[-1, QT]],
                                    )
                                else:  # causal edge: qi-jj >= 128kke-512
                                    nc.gpsimd.affine_select(
                                        out=pt[:], in_=praw[:],
                                        compare_op=mybir.AluOpType.is_ge,
                                        fill=0.0, base=WINDOW - 128 * kke,
                                        channel_multiplier=-1,
                                        pattern=[[1, QT]],
                                    )
                            else:
                                nc.vector.tensor_mul(
                                    pt[:], praw[:],
                                    mask_sb[:, ds(896 - 128 * kke, QT)],
                                )
                            pts.append(pt)

                        def pv(idx):
                            j0, _ = chunks[idx]
                            nc.tensor.matmul(ops[:], Vn[b][:, ds(j0, 128)],
                                             pts[idx][:], start=(idx == 0),
                                             stop=(idx == n - 1))
                            if idx % 2 == 1:
                                # denominator on summed pt pairs: half the
                                # ones-matmuls at the cost of one DVE add
                                pta = sb.tile([128, QT], BF16, tag="pta",
                                              bufs=3, name="pta")
                                nc.vector.tensor_add(pta[:], pts[idx - 1][:],
                                                     pts[idx][:])
                                nc.tensor.matmul(lps[:], ones_sb[:], pta[:],
                                                 start=(idx == 1),
                                                 stop=(idx == n - 1))

                        score_chain(0)
                        for idx in range(1, n):
                            score_chain(idx)
                            pv(idx - 1)
                        pv(n - 1)
                        linv = sb.tile([1, QT], F32, tag="linv", bufs=2,
                                       name="linv")
                        nc.vector.reciprocal_approx_fast(linv[:], lps[:])
                        lbc = sb.tile([128, QT], F32, tag="lbc", bufs=2,
                                      name="lbc")
                        nc.gpsimd.partition_broadcast(lbc[:], linv[:])
                        nc.vector.tensor_mul(a[:, ds(i0, QT)], ops[:], lbc[:])

        def oproj(b, wo_sb):
            with nc.named_scope(f"oproj_b{b}"):
                trig = {r0 + nr: (k, r0, nr)
                        for k, (r0, nr) in enumerate(RS_CHUNKS[b])}
                for o in range(NOC):
                    # one [128, S] staging tile per of-chunk -> single 512KB
                    # DMA with 4KB contiguous lines
                    ot = sb.tile([128, S], BF16, tag="ot", bufs=4, name="ot")
                    for tk in range(S // 512):
                        i = o * 4 + tk
                        po = ps.tile([128, 512], F32, tag=f"pb{4 + i % 2}",
                                     bufs=1, name="po")
                        for h in range(HPC):
                            idx = h * NOC + o
                            nc.tensor.matmul(
                                po[:],
                                wo_sb[:, idx // 4, ds((idx % 4) * 128, 128)],
                                asb[(b, h)][:, ts(tk, 512)],
                                start=(h == 0), stop=(h == HPC - 1),
                            )
                        if i % 2 == 0:
                            nc.scalar.copy(ot[:, ts(tk, 512)], po[:])
                        else:
                            nc.vector.tensor_copy(ot[:, ts(tk, 512)], po[:])
                    nc.sync.dma_start(rsin[b][ds(o * 128, 128), :], ot[:])
                    if (o + 1) * 128 in trig:
                        k, r0, nr = trig[(o + 1) * 128]
                        nc.gpsimd.collective_compute(
                            "ReduceScatter",
                            mybir.AluOpType.add,
                            replica_groups=rg,
                            ins=[rsin[b][ds(r0, nr), :]],
                            outs=[rsout[b][k][:]],
                        )

        def outcopy(b):
            orow = 0
            for k, (r0, nr) in enumerate(RS_CHUNKS[b]):
                pr = nr // NC
                for r in range(pr // 128):
                    t = sb.tile([128, S], BF16, tag="oc", bufs=2, name="oc")
                    nc.sync.dma_start(t[:], rsout[b][k][ds(r * 128, 128), :])
                    nc.sync.dma_start(out[b, ds(orow, 128), :], t[:])
                    orow += 128

        proj(0)
        proj(1)
        # wo load rides the DMA-free attention phase; reuses wq's SBUF slot
        # (wq is dead once proj(1) has drained)
        wo_sb = sb.tile([128, FCH, QW], BF16, tag="wq", bufs=1, name="wo_sb")
        nc.sync.dma_start(wo_sb[:], wo)
        attn(0)
        oproj(0, wo_sb)
        attn(1)
        outcopy(0)
        oproj(1, wo_sb)
        outcopy(1)

    nc.compile()
    return nc


def _prep_inputs(inputs):
    hidden = np.asarray(inputs["hidden_states"], np.float32)
    pos = np.asarray(inputs["position_ids"])
    cos = np.asarray(inputs["cos"], np.float32)
    sin = np.asarray(inputs["sin"], np.float32)
    wq = np.asarray(inputs["wq"], np.float32)
    wk = np.asarray(inputs["wk"], np.float32)
    wv = np.asarray(inputs["wv"], np.float32)
    wo = np.asarray(inputs["wo"], np.float32)
    qw = np.asarray(inputs["q_norm_w"], np.float32)
    kw = np.asarray(inputs["k_norm_w"], np.float32)

    hT = np.ascontiguousarray(hidden.transpose(0, 2, 1)).astype(npbf16)
    cosT = np.ascontiguousarray(cos[pos].transpose(0, 2, 1)).astype(npbf16)
    sinT_f = sin[pos].transpose(0, 2, 1).copy()
    sinT_f[:, 0::2, :] *= -1.0
    sinT = np.ascontiguousarray(sinT_f).astype(npbf16)

    winvq = (1.0 / np.where(qw == 0, 1, qw) ** 2).astype(np.float32).reshape(D, 1)
    winvk = (1.0 / np.where(kw == 0, 1, kw) ** 2).astype(np.float32).reshape(D, 1)

    in_maps = []
    for c in range(NC):
        wq_c = wq[:, c * QW:(c + 1) * QW].copy()
        for j in range(HPC):
            blk = wq_c[:, j * D:(j + 1) * D]
            blk -= blk.mean(axis=1, keepdims=True)
            blk *= qw[None, :]
        wk_c = wk[:, c * D:(c + 1) * D].copy()
        wk_c -= wk_c.mean(axis=1, keepdims=True)
        wk_c *= kw[None, :]
        wo_c = wo[c * QW:(c + 1) * QW, :]
        wo_t = np.ascontiguousarray(
            wo_c.reshape(HPC, 128, NOC, 128).transpose(1, 0, 2, 3)
            .reshape(128, FCH, QW)
        ).astype(npbf16)
        in_maps.append({
            "hT": hT,
            "cosT": cosT,
            "sinT": sinT,
            "wq": np.ascontiguousarray(wq_c).astype(npbf16),
            "wk": np.ascontiguousarray(wk_c).astype(npbf16),
            "wv": np.ascontiguousarray(wv[:, c * D:(c + 1) * D]).astype(npbf16),
            "wo": wo_t,
            "winvq": winvq,
            "winvk": winvk,
        })
    return in_maps


def _run(inputs, **kwargs):
    if "nc" not in _CACHE:
        _CACHE["nc"] = _build_module()
    nc = _CACHE["nc"]
    in_maps = _prep_inputs(inputs)
    res = run_bass_kernel_spmd(nc, in_maps, core_ids=list(range(NC)), **kwargs)
    # core c returns out[b] = concat over RS chunks k of of-rows
    # [r0 + c*nr/8, r0 + (c+1)*nr/8) of outT[b]
    outT = np.empty((B, HID, S), np.float32)
    for c in range(NC):
        shard = np.asarray(res.results[c]["out"], dtype=np.float32)
        for b in range(B):
            orow = 0
            for r0, nr in RS_CHUNKS[b]:
                pr = nr // NC
                outT[b, r0 + c * pr: r0 + (c + 1) * pr, :] = \
                    shard[b, orow: orow + pr]
                orow += pr
    full = np.ascontiguousarray(outT.transpose(0, 2, 1))
    return full, res


def kernel(**inputs) -> np.ndarray:
    out, _ = _run(inputs)
    return out


if __name__ == "__main__":
    import reference
    ins = {k: np.asarray(v) for k, v in reference.setup_inputs().items()}
    expected = np.asarray(reference.reference(**reference.setup_inputs()))
    actual = kernel(**ins)
    err = np.linalg.norm(actual - expected) / np.linalg.norm(expected)
    print("Relative error:", err)
